# revision 35
# baseline (speedup 1.0000x reference)
"""AnisotropySuppressionLoss on 8 TRN2 NeuronCores (Bass/Tile).

Quadrant-folded real-input 2D DFT in bf16, now folded along BOTH axes before
step 1: the host ships the image as bf16 in two row blocks (rows 0..512, and
rows 1023..513 reversed so the row-fold r -> 1024-r is an aligned tile
add/sub).  Step 1 then contracts over only 513 rows (4 chunks + 1 row)
instead of 1024 -- half the MACs and half the input DMA of the previous
version.

With CE/SO/CE2/SO2 the four step-2 partial sums (cos/sin x even/odd folds):
  Fr(v) = CE+SO, Fr(1024-v) = CE-SO, Fi(v) = CE2-SO2, Fi(1024-v) = CE2+SO2.
With sqrt(2*w_u) baked into the trig matrices:
  SQ := CE^2+SO^2+CE2^2+SO2^2  equals the quadrant fold w_u*(P(v)+P(1024-v)),
  X  := CE*SO - CE2*SO2,  and  Plo^2+Phi^2 = SQ^2/2 + 2*X^2
so the radial-bin input G is just SQ (cols 0/512 halved) and the p2 term
accumulates from SQ and X row-reductions. Radial segment sums use the exact
barrel-free local_scatter; loss_img = sum_w P^2 - sum_k S_k^2/c_k.

Schedule: chunk-granular DMA -> fold -> step-1 pipeline (matmuls start as
soon as the first 128-row chunk lands); step 2 runs v-chunks in descending
order so the diagonal fold + scatters + radial reductions stream out behind
it; image 1's step-1 groups are interleaved between image 0's step-2 groups
to keep the PE dense (HAM stays warm).  Data-parallel: batch 16 -> 2 images
per core; host averages.
"""

import os
import sys

sys.path.insert(0, "/opt/trn_rl_repo")

import numpy as np

import concourse.bass as bass
import concourse.tile as tile
from concourse import bacc, mybir
from concourse.bass_utils import run_bass_kernel_spmd
from concourse.masks import make_identity

F32 = mybir.dt.float32
F32R = mybir.dt.float32r
BF16 = mybir.dt.bfloat16

H = 1024
NQ = 513          # quadrant size (0..512 per axis)
NB = 725          # radial bins 0..724
WB = 728          # bin buffer width
N_CORES = 8
IMGS_PER_CORE = 2
WA = 0.002
EPS = 1e-12
CHUNKS = [(0, 128), (128, 256), (256, 384), (384, 512), (512, 513)]
# per-chunk scatter windows: chunk ci rows have bins in [floor(sqrt(2)*c0), 725)
SCAT = [(0, 514, 0, 726), (128, 386, 181, 546), (256, 258, 362, 364),
        (384, 130, 543, 184), (512, 2, 724, 2)]
FAMS = ("ee", "oe", "eo", "oo")

_CACHE = {}


# ---------------------------------------------------------------- host consts
def _host_constants():
    if "consts" in _CACHE:
        return _CACHE["consts"]
    import ml_dtypes

    cu = np.arange(NQ, dtype=np.float64)
    wu = np.full(NQ, 2.0)
    wu[0] = 1.0
    wu[512] = 1.0
    sqw = np.sqrt(wu) / 32.0          # step-1 col scale: sqrt(w_u), half of 1/H
    s2c = np.sqrt(2.0) / 32.0         # step-2 scale (makes SQ == G directly)

    # step-1 trig with the r -> 1024-r fold baked in: contract r = 0..512.
    rr = np.arange(NQ, dtype=np.float64)
    angf = 2.0 * np.pi * np.outer(rr, cu) / H
    Ct = np.cos(angf) * sqw[None, :]          # [513, 513] rows r, cols v
    St = (-np.sin(angf) * sqw[None, :])[:512]  # [512, 513]; row 0 is zero

    ang = 2.0 * np.pi * np.outer(cu, cu) / H
    C2 = np.cos(ang) * s2c
    C2[512] *= 0.5                    # xe col 512 is 2x true; compensate
    S2 = (np.sin(ang) * s2c)[:512]

    # p2 per-partition weights: cols 2mu = 0.5/w_u, 2mu+1 = 2/w_u
    wc = np.zeros((128, 10), dtype=np.float32)
    for mu in range(5):
        c0, c1 = CHUNKS[mu]
        n = c1 - c0
        wc[:n, 2 * mu] = (0.5 / wu[c0:c1]).astype(np.float32)
        wc[:n, 2 * mu + 1] = (2.0 / wu[c0:c1]).astype(np.float32)

    # radial bin counts exactly as reference._radial_bins (unshifted coords)
    y = np.minimum(np.arange(H), H - np.arange(H))
    yy, xx = np.meshgrid(y, y, indexing="ij")
    dist = np.sqrt((xx.astype(np.float64)) ** 2 + yy.astype(np.float64) ** 2)
    bins_full = np.clip(dist.astype(np.int32), 0, NB - 1)
    counts = np.bincount(bins_full.reshape(-1), minlength=NB).astype(np.float64)
    invc = np.zeros((33, WB), dtype=np.float32)
    invc[0, :NB] = (1.0 / counts).astype(np.float32)
    invc[32, :NB] = invc[0, :NB]

    # per-(chunk,parity) scatter indices: cell (u, j>=u) -> bin floor(sqrt(u^2+j^2));
    # within a row and j-parity, bins are strictly increasing (dr/dj >= 1/sqrt(2))
    # so each local_scatter sees no duplicate indices. -1 = masked.
    NI = 514
    ie = -np.ones((640, NI), dtype=np.int16)
    io = -np.ones((640, NI), dtype=np.int16)
    for ci in range(5):
        c0, c1 = CHUNKS[ci]
        b0 = SCAT[ci][2]
        for p in range(c1 - c0):
            u = c0 + p
            js = np.arange(u, NQ)
            ks = np.floor(np.sqrt(u * u + js.astype(np.float64) ** 2)).astype(np.int16)
            tgt_e = js % 2 == 0
            ie[128 * ci + p, js[tgt_e] - c0] = ks[tgt_e] - b0
            io[128 * ci + p, js[~tgt_e] - c0] = ks[~tgt_e] - b0

    # full-width pre-masks: 0 below diag, 0.5 on diag (doubled by fold),
    # 1 above; chunk 4 keeps its single diagonal cell at weight 1.
    mfull = np.zeros((640, NQ), dtype=np.float32)
    for ci in range(4):
        a = 128 * ci + np.arange(128)
        cols = np.arange(NQ)
        blk = (cols[None, :] > a[:, None]).astype(np.float32)
        blk[np.arange(128), a] = 0.5
        mfull[128 * ci : 128 * ci + 128] = blk
    mfull[512, 512] = 1.0

    bf = ml_dtypes.bfloat16
    _CACHE["consts"] = dict(
        ct=Ct.astype(bf), st=St.astype(bf), c2=C2.astype(bf), s2=S2.astype(bf),
        wc=wc, invc=invc, ie=ie, io=io,
        mfull_bf16=mfull.astype(bf), counts=counts,
    )
    return _CACHE["consts"]


# ---------------------------------------------------------------- device build
def _build_nc():
    nc = bacc.Bacc("TRN2", target_bir_lowering=False, debug=False)
    xlo_p = nc.declare_dram_parameter("xlo", [IMGS_PER_CORE, NQ, H], BF16, isOutput=False)
    xhr_p = nc.declare_dram_parameter("xhr", [IMGS_PER_CORE, NQ, H], BF16, isOutput=False)
    ct_p = nc.declare_dram_parameter("ct", [NQ, NQ], BF16, isOutput=False)
    st_p = nc.declare_dram_parameter("st", [512, NQ], BF16, isOutput=False)
    c2_p = nc.declare_dram_parameter("c2", [NQ, NQ], BF16, isOutput=False)
    s2_p = nc.declare_dram_parameter("s2", [512, NQ], BF16, isOutput=False)
    ie_p = nc.declare_dram_parameter("ie", [640, 514], mybir.dt.int16, isOutput=False)
    io_p = nc.declare_dram_parameter("io", [640, 514], mybir.dt.int16, isOutput=False)
    mf_p = nc.declare_dram_parameter("mf", [640, NQ], BF16, isOutput=False)
    wc_p = nc.declare_dram_parameter("wc", [128, 10], F32, isOutput=False)
    ic_p = nc.declare_dram_parameter("ic", [33, WB], F32, isOutput=False)
    out_p = nc.declare_dram_parameter("out", [1, IMGS_PER_CORE], F32, isOutput=True)

    AT = mybir.AluOpType
    AF = mybir.ActivationFunctionType

    with tile.TileContext(nc) as tc:
        with (
            tc.tile_pool(name="const", bufs=1) as cpool,
            tc.tile_pool(name="xin", bufs=2) as xpool,
            tc.tile_pool(name="fold", bufs=2) as fpool,
            tc.tile_pool(name="amat", bufs=2) as apool,
            tc.tile_pool(name="quad", bufs=2) as qpool,
            tc.tile_pool(name="dedo", bufs=2) as dpool,
            tc.tile_pool(name="work", bufs=2) as wpool,
            tc.tile_pool(name="ps", bufs=2, space="PSUM") as ps,
            tc.tile_pool(name="ps1", bufs=1, space="PSUM") as ps1,
        ):
            # ---------------- constant tiles
            ct_t = [cpool.tile([128, NQ], BF16, tag=f"ct{k}", name=f"ct{k}") for k in range(4)]
            st_t = [cpool.tile([128, NQ], BF16, tag=f"st{k}", name=f"st{k}") for k in range(4)]
            ct512 = cpool.tile([1, NQ], BF16, tag="ct512", name="ct512")
            c2_t = [cpool.tile([128, NQ], BF16, tag=f"c2{k}", name=f"c2{k}") for k in range(4)]
            s2_t = [cpool.tile([128, NQ], BF16, tag=f"s2{k}", name=f"s2{k}") for k in range(4)]
            c2x_t = cpool.tile([1, NQ], BF16, tag="c2x", name="c2x")
            ie_t = [cpool.tile([128, 514], mybir.dt.int16, tag=f"ie{ci}", name=f"ie{ci}")
                    for ci in range(5)]
            io_t = [cpool.tile([128, 514], mybir.dt.int16, tag=f"io{ci}", name=f"io{ci}")
                    for ci in range(5)]
            mf_t = [cpool.tile([128, NQ], BF16, tag=f"mf{ci}", name=f"mf{ci}") for ci in range(5)]
            wc_t = cpool.tile([128, 10], F32, tag="wc", name="wc")
            ic_t = cpool.tile([33, WB], F32, tag="ic", name="ic")
            ident = cpool.tile([128, 128], F32, tag="ident", name="ident")
            ones32 = cpool.tile([128, 1], F32, tag="ones32", name="ones32")
            ones = cpool.tile([128, 1], F32R, tag="ones", name="ones")
            onesb = cpool.tile([128, 1], BF16, tag="onesb", name="onesb")
            zt = cpool.tile([128, WB], F32, tag="zt", name="zt")
            lossv = cpool.tile([1, IMGS_PER_CORE], F32, tag="lossv", name="lossv")

            # ---------------- DMA helpers (emission order == queue order)
            def dma_round(img, k):
                lo = xpool.tile([128, H], BF16, tag="lo", bufs=3, name=f"lo{k}")
                hi = xpool.tile([128, H], BF16, tag="hi", bufs=3, name=f"hi{k}")
                nc.sync.dma_start(lo[:], xlo_p[img, 128 * k : 128 * k + 128, :])
                nc.sync.dma_start(hi[:], xhr_p[img, 128 * k : 128 * k + 128, :])
                return lo, hi

            def dma_row512(img):
                lo5 = xpool.tile([1, H], BF16, tag="lo5", name="lo5")
                nc.sync.dma_start(lo5[:], xlo_p[img, 512:513, :])
                return lo5

            # ---------------- fold: r-fold (tile add) then c-fold (in-row)
            fq = {}    # fq[img][fam][k] fold tiles; fq5[img][fam] row-512
            fq5 = {}

            def fold_chunk(img, k, lo, hi):
                xE = fpool.tile([128, H], BF16, tag="xE", name=f"xE{k}")
                xO = fpool.tile([128, H], BF16, tag="xO", name=f"xO{k}")
                nc.vector.tensor_tensor(out=xE[:], in0=lo[:], in1=hi[:], op=AT.add)
                nc.vector.tensor_tensor(out=xO[:], in0=lo[:], in1=hi[:], op=AT.subtract)
                out = {}
                for fam, src in (("ee", xE), ("eo", xE), ("oe", xO), ("oo", xO)):
                    f = fpool.tile([128, NQ], BF16, tag=f"f{fam}{k}", name=f"f{fam}{k}")
                    even = fam in ("ee", "oe")
                    eng = nc.vector if even else nc.gpsimd
                    eng.tensor_tensor(
                        out=f[:, 1:513], in0=src[:, 1:513],
                        in1=src[:, 1023:511:-1],
                        op=AT.add if even else AT.subtract,
                    )
                    if even:
                        nc.scalar.activation(f[:, 0:1], src[:, 0:1], AF.Copy)
                    else:
                        nc.gpsimd.memset(f[:, 0:1], 0.0)
                    out[fam] = f
                fq.setdefault(img, {f: [None] * 4 for f in FAMS})
                for fam in FAMS:
                    fq[img][fam][k] = out[fam]

            def fold_row512(img, lo5):
                # r = 512: xE row = x[512] (xhr row 512 is zero), xO row = 0.
                # only cos families need it (sin row 512 is zero).
                d = {}
                for fam, op in (("ee", AT.add), ("eo", AT.subtract)):
                    f = fpool.tile([1, NQ], BF16, tag=f"f5{fam}", name=f"f5{fam}")
                    nc.vector.tensor_tensor(
                        out=f[0:1, 1:513], in0=lo5[0:1, 1:513],
                        in1=lo5[0:1, 1023:511:-1], op=op,
                    )
                    if fam == "ee":
                        nc.scalar.activation(f[0:1, 0:1], lo5[0:1, 0:1], AF.Copy)
                    else:
                        nc.scalar.activation(f[0:1, 0:1], zt[0:1, 0:1], AF.Copy)
                    d[fam] = f
                fq5[img] = d

            # ---------------- step 1: A = fold.T @ trig  (contract 513 rows)
            A_t = {}   # A_t[img][fam][m] [128,512] bf16
            Acol = {}  # Acol[img][fam] [128,4]
            A512 = {}  # A512[img][fam] [1,NQ]

            def s1_alloc(img):
                A_t[img] = {fam: [apool.tile([128, 512], BF16, tag=f"A{fam}{m}",
                                             name=f"A{fam}{m}") for m in range(4)]
                            for fam in FAMS}
                Acol[img] = {fam: apool.tile([128, 4], BF16, tag=f"Ac{fam}", name=f"Ac{fam}")
                             for fam in ("ee", "eo")}
                A512[img] = {fam: apool.tile([1, NQ], BF16, tag=f"A5{fam}", name=f"A5{fam}")
                             for fam in ("ee", "oe")}

            def s1_group(img, fam, m):
                # one (fam, u-chunk) output: psA [128,512] over 4(+1) r-chunks
                cos_fam = fam in ("ee", "eo")
                rhsM = ct_t if cos_fam else st_t
                xf = fq[img][fam]
                psA = ps.tile([128, 512], F32, tag="pbig", name="pbig")
                if cos_fam:
                    psAc = ps.tile([128, 16], F32, tag="psml", bufs=1, name="psml")
                for k in range(4):
                    st_, sp = (k == 0), (k == 3 and not cos_fam)
                    lhs = xf[k][:, 128 * m : 128 * m + 128]
                    nc.tensor.matmul(psA[:], lhs, rhsM[k][:, 0:512], start=st_, stop=sp)
                    if cos_fam:
                        nc.tensor.matmul(psAc[:, 0:1], lhs, rhsM[k][:, 512:513],
                                         start=st_, stop=False)
                if cos_fam:
                    lhs5 = fq5[img][fam][0:1, 128 * m : 128 * m + 128]
                    nc.tensor.matmul(psA[:], lhs5, ct512[0:1, 0:512],
                                     start=False, stop=True, skip_group_check=True)
                    nc.tensor.matmul(psAc[:, 0:1], lhs5, ct512[0:1, 512:513],
                                     start=False, stop=True, skip_group_check=True)
                nc.scalar.activation(A_t[img][fam][m][:], psA[:], AF.Copy)
                if cos_fam:
                    nc.scalar.activation(Acol[img][fam][:, m : m + 1],
                                         psAc[:, 0:1], AF.Copy)

            def s1_a512(img, fam):
                # u = 512 row of A for fams ee (cos) / oe (sin): M=1 matmuls
                cos_fam = fam == "ee"
                rhsM = ct_t if cos_fam else st_t
                xf = fq[img]["ee" if cos_fam else "oe"]
                psRf = ps.tile([128, 512], F32, tag="pbig", name="pbig")
                psR = psRf[0:1, :]
                psRcf = ps.tile([128, 16], F32, tag="psml", bufs=1, name="psml")
                psRc = psRcf[0:1, 0:1]
                for k in range(4):
                    st_, sp = (k == 0), (k == 3 and not cos_fam)
                    lhs = xf[k][:, 512:513]
                    nc.tensor.matmul(psR, lhs, rhsM[k][:, 0:512], start=st_, stop=sp)
                    if cos_fam:
                        nc.tensor.matmul(psRc, lhs, rhsM[k][:, 512:513],
                                         start=st_, stop=False)
                if cos_fam:
                    lhs5 = fq5[img]["ee"][0:1, 512:513]
                    nc.tensor.matmul(psR, lhs5, ct512[0:1, 0:512],
                                     start=False, stop=True, skip_group_check=True)
                    nc.tensor.matmul(psRc, lhs5, ct512[0:1, 512:513],
                                     start=False, stop=True, skip_group_check=True)
                    nc.scalar.activation(A512[img]["ee"][0:1, 0:512], psR, AF.Copy)
                    nc.scalar.activation(A512[img]["ee"][0:1, 512:513], psRc, AF.Copy)
                else:
                    nc.scalar.activation(A512[img]["oe"][0:1, 0:512], psR, AF.Copy)
                    # sin col 512 is exactly zero
                    nc.scalar.activation(A512[img]["oe"][0:1, 512:513], zt[0:1, 0:1],
                                         AF.Copy)

            # ---------------- step 2 state
            Gt = {}
            p2 = {}

            def s2_alloc(img):
                Gt[img] = [qpool.tile([128, 516], F32, tag=f"g{ci}", name=f"g{ci}")
                           for ci in range(5)]
                p2[img] = qpool.tile([128, 8], F32R, tag="p2acc", name="p2acc")
                nc.vector.tensor_copy(p2[img][:, 0:8], zt[:, 0:8])

            s2x = {}

            def s2_mu(img, mu, defer_x=False):
                p2acc = p2[img]
                At = A_t[img]
                M = 128
                u0 = 128 * mu
                if img == 0:
                    psCE = ps1.tile([128, 512], F32, tag="p2ce", name="p2ce")
                    psCE2 = ps1.tile([128, 512], F32, tag="p2ce2", name="p2ce2")
                else:
                    psCE = ps.tile([128, 512], F32, tag="pbig", name="pbig")
                    psCE2 = ps.tile([128, 512], F32, tag="pbig", name="pbig")
                psSO = ps1.tile([128, 512], F32, tag="p2so", name="p2so")
                psSO2 = ps1.tile([128, 512], F32, tag="p2so2", name="p2so2")
                pscc = ps.tile([128, 16], F32, tag="psml", bufs=1, name="psml")
                psc = pscc[:, 0:8]
                psc2 = pscc[:, 8:16]
                for k in range(4):
                    st_ = (k == 0)
                    lee = At["ee"][k][:, u0 : u0 + M]
                    loe = At["oe"][k][:, u0 : u0 + M]
                    nc.tensor.matmul(psCE[0:M], lee, c2_t[k][:, 0:512],
                                     start=st_, stop=False)
                    nc.tensor.matmul(psc[0:M, 0:1], lee, c2_t[k][:, 512:513],
                                     start=st_, stop=False)
                    nc.tensor.matmul(psCE2[0:M], loe, c2_t[k][:, 0:512],
                                     start=st_, stop=False)
                    nc.tensor.matmul(psc2[0:M, 0:1], loe, c2_t[k][:, 512:513],
                                     start=st_, stop=False, skip_group_check=True)
                xee = A512[img]["ee"][0:1, u0 : u0 + M]
                xoe = A512[img]["oe"][0:1, u0 : u0 + M]
                nc.tensor.matmul(psCE[0:M], xee, c2x_t[0:1, 0:512],
                                 start=False, stop=True, skip_group_check=True)
                nc.tensor.matmul(psc[0:M, 0:1], xee, c2x_t[0:1, 512:513],
                                 start=False, stop=True, skip_group_check=True)
                nc.tensor.matmul(psCE2[0:M], xoe, c2x_t[0:1, 0:512],
                                 start=False, stop=True, skip_group_check=True)
                nc.tensor.matmul(psc2[0:M, 0:1], xoe, c2x_t[0:1, 512:513],
                                 start=False, stop=True, skip_group_check=True)
                for k in range(4):
                    st_ = (k == 0)
                    loo = At["oo"][k][:, u0 : u0 + M]
                    leo = At["eo"][k][:, u0 : u0 + M]
                    nc.tensor.matmul(psSO[0:M], loo, s2_t[k][:, 0:512],
                                     start=st_, stop=(k == 3))
                    nc.tensor.matmul(psSO2[0:M], leo, s2_t[k][:, 0:512],
                                     start=st_, stop=(k == 3))

                # square-during-evacuation on scalar (PSUM f32 -> bf16 squares);
                # the X cross-products read PSUM directly on vector.
                sqa = wpool.tile([128, 512], BF16, tag="sqa", name="sqa")
                sqb = wpool.tile([128, 512], BF16, tag="sqb", name="sqb")
                sqc = wpool.tile([128, 512], BF16, tag="sqc", name="sqc")
                sqd = wpool.tile([128, 512], BF16, tag="sqd", name="sqd")
                sqe = wpool.tile([128, 2], F32, tag="sqe", name="sqe")
                def emit_x_products():
                    q1 = wpool.tile([128, 512], BF16, tag="q1", name="q1")
                    q2w = wpool.tile([128, 512], BF16, tag="q2w", name="q2w")
                    sos = wpool.tile([128, 512], BF16, tag="sos", name="sos")
                    so2s = wpool.tile([128, 512], BF16, tag="so2s", name="so2s")
                    nc.vector.tensor_copy(sos[0:M], psSO[0:M])
                    nc.vector.tensor_copy(so2s[0:M], psSO2[0:M])
                    nc.vector.tensor_tensor(out=q1[0:M], in0=sos[0:M],
                                            in1=psCE[0:M], op=AT.mult)
                    nc.vector.tensor_tensor(out=q2w[0:M], in0=so2s[0:M],
                                            in1=psCE2[0:M], op=AT.mult)
                    return q1, q2w

                def emit_x_tail(q1, q2w):
                    xp = wpool.tile([128, 512], BF16, tag="xp", bufs=1, name="xp")
                    nc.vector.tensor_tensor(out=xp[0:M], in0=q1[0:M],
                                            in1=q2w[0:M], op=AT.subtract)
                    junk2 = wpool.tile([128, 513], BF16, tag="junk", bufs=1,
                                       name="junk")
                    xrs = wpool.tile([128, 1], F32, tag="xrs", name="xrs")
                    nc.scalar.activation(junk2[0:M, 0:512], xp[0:M], AF.Square,
                                         accum_out=xrs[0:M])
                    nc.vector.scalar_tensor_tensor(
                        p2acc[0:M, 0:1], xrs[0:M],
                        wc_t[0:M, 2 * mu + 1 : 2 * mu + 2],
                        p2acc[0:M, 0:1], op0=AT.mult, op1=AT.add,
                    )

                if defer_x:
                    s2x[img] = (emit_x_products, emit_x_tail)
                else:
                    _q1, _q2w = emit_x_products()
                nc.scalar.activation(sqa[0:M], psCE[0:M], AF.Square)
                nc.scalar.activation(sqb[0:M], psSO[0:M], AF.Square)
                nc.scalar.activation(sqc[0:M], psCE2[0:M], AF.Square)
                nc.scalar.activation(sqd[0:M], psSO2[0:M], AF.Square)
                nc.scalar.activation(sqe[0:M, 0:2], pscc[0:M, 0:16:8], AF.Square,
                                     scale=float(np.sqrt(0.5)))
                s1t = wpool.tile([128, 512], BF16, tag="s1t", bufs=1, name="s1t")
                s2w = wpool.tile([128, 512], BF16, tag="s2w", bufs=1, name="s2w")
                nc.vector.tensor_tensor(out=s1t[0:M], in0=sqa[0:M], in1=sqb[0:M],
                                        op=AT.add)
                nc.vector.tensor_tensor(out=s2w[0:M], in0=sqc[0:M], in1=sqd[0:M],
                                        op=AT.add)
                G = Gt[img][mu]
                nc.vector.tensor_tensor(out=G[0:M, 0:512], in0=s1t[0:M],
                                        in1=s2w[0:M], op=AT.add)
                nc.vector.tensor_tensor(out=G[0:M, 512:513], in0=sqe[0:M, 0:1],
                                        in1=sqe[0:M, 1:2], op=AT.add)
                nc.vector.tensor_scalar_mul(G[0:M, 0:1], G[0:M, 0:1], 0.5)
                if mu == 0:
                    nc.vector.tensor_scalar_mul(G[0:1, 0:1], G[0:1, 0:1], 0.0)
                junk = wpool.tile([128, 513], BF16, tag="junk", bufs=1, name="junk")
                srs = wpool.tile([128, 1], F32, tag="srs", name="srs")
                crs = wpool.tile([128, 1], F32, tag="crs", name="crs")
                nc.scalar.activation(junk[0:M, 0:513], G[0:M, 0:513],
                                     AF.Square, accum_out=srs[0:M])
                nc.scalar.activation(junk[0:M, 0:2], G[0:M, 0:513:512],
                                     AF.Square, accum_out=crs[0:M])
                nc.vector.scalar_tensor_tensor(
                    p2acc[0:M, 0:1], srs[0:M], wc_t[0:M, 2 * mu : 2 * mu + 1],
                    p2acc[0:M, 0:1], op0=AT.mult, op1=AT.add,
                )
                nc.vector.scalar_tensor_tensor(
                    p2acc[0:M, 0:1], crs[0:M], wc_t[0:M, 2 * mu : 2 * mu + 1],
                    p2acc[0:M, 0:1], op0=AT.mult, op1=AT.add,
                )
                if not defer_x:
                    emit_x_tail(_q1, _q2w)

            def s2_x_run(img):
                prod, tail = s2x.pop(img)
                tail(*prod())

            def s2_u512(img):
                # u = 512 (Nyquist) row: SO and CE2 are exactly 0, X = 0
                p2acc = p2[img]
                psCE = ps1.tile([128, 512], F32, tag="p2ce", name="p2ce")
                psSO2 = ps1.tile([128, 512], F32, tag="p2so2", name="p2so2")
                pscf = ps.tile([128, 16], F32, tag="psml", bufs=1, name="psml")
                psc = pscf[:, 0:8]
                for k in range(4):
                    st_ = (k == 0)
                    nc.tensor.matmul(psCE[0:1], Acol[img]["ee"][:, k : k + 1],
                                     c2_t[k][:, 0:512], start=st_, stop=False)
                    nc.tensor.matmul(psc[0:1, 0:1], Acol[img]["ee"][:, k : k + 1],
                                     c2_t[k][:, 512:513], start=st_, stop=False)
                    nc.tensor.matmul(psSO2[0:1], Acol[img]["eo"][:, k : k + 1],
                                     s2_t[k][:, 0:512], start=st_, stop=(k == 3))
                nc.tensor.matmul(psCE[0:1], A512[img]["ee"][0:1, 512:513],
                                 c2x_t[0:1, 0:512],
                                 start=False, stop=True, skip_group_check=True)
                nc.tensor.matmul(psc[0:1, 0:1], A512[img]["ee"][0:1, 512:513],
                                 c2x_t[0:1, 512:513],
                                 start=False, stop=True, skip_group_check=True)
                sqe = wpool.tile([128, 2], F32, tag="sqe", name="sqe")
                sqa = wpool.tile([128, 512], BF16, tag="sqa", name="sqa")
                sqd = wpool.tile([128, 512], BF16, tag="sqd", name="sqd")
                nc.scalar.activation(sqa[0:1], psCE[0:1], AF.Square)
                nc.scalar.activation(sqd[0:1], psSO2[0:1], AF.Square)
                nc.scalar.activation(sqe[0:1, 0:1], psc[0:1, 0:1], AF.Square,
                                     scale=float(np.sqrt(0.5)))
                G4 = Gt[img][4]
                nc.vector.tensor_tensor(out=G4[0:1, 0:512], in0=sqa[0:1],
                                        in1=sqd[0:1], op=AT.add)
                nc.vector.tensor_copy(G4[0:1, 512:513], sqe[0:1, 0:1])
                nc.vector.tensor_scalar_mul(G4[0:1, 0:1], G4[0:1, 0:1], 0.5)
                junk = wpool.tile([128, 513], BF16, tag="junk", bufs=1, name="junk")
                srs = wpool.tile([128, 1], F32, tag="srs", name="srs")
                crs = wpool.tile([128, 1], F32, tag="crs", name="crs")
                crs2 = wpool.tile([128, 1], F32, tag="crs2", name="crs2")
                nc.scalar.activation(junk[0:1, 0:513], G4[0:1, 0:513],
                                     AF.Square, accum_out=srs[0:1])
                nc.scalar.activation(junk[0:1, 0:1], G4[0:1, 0:1],
                                     AF.Square, accum_out=crs[0:1])
                nc.scalar.activation(junk[0:1, 1:2], G4[0:1, 512:513],
                                     AF.Square, accum_out=crs2[0:1])
                for acc in (srs, crs, crs2):
                    nc.vector.scalar_tensor_tensor(
                        p2acc[0:1, 0:1], acc[0:1], wc_t[0:1, 8:9],
                        p2acc[0:1, 0:1], op0=AT.mult, op1=AT.add,
                    )

            # ---------------- diagonal fold + scatter + radial reduce
            de_t = {}
            do_t = {}

            def df_alloc(img):
                de_t[img] = [dpool.tile([128 if ci < 4 else 16, SCAT[ci][3]], BF16,
                                        tag=f"de{ci}", name=f"de{ci}")
                             for ci in range(5)]
                do_t[img] = [dpool.tile([128 if ci < 4 else 16, SCAT[ci][3]], BF16,
                                        tag=f"do{ci}", name=f"do{ci}")
                             for ci in range(5)]

            def scat(img, ci):
                ch = 128 if ci < 4 else 16
                c0, ni, b0, ne = SCAT[ci]
                gd = wpool.tile([128, 514], BF16, tag=f"gd{ci}", bufs=1,
                                name=f"gd{ci}")
                nc.vector.tensor_tensor(
                    out=gd[:, c0:NQ], in0=Gt[img][ci][:, c0:NQ],
                    in1=mf_t[ci][:, c0:NQ], op=AT.mult,
                )
                nc.gpsimd.local_scatter(
                    de_t[img][ci][0:ch, :], gd[0:ch, c0 : c0 + ni],
                    ie_t[ci][0:ch, 0:ni],
                    channels=ch, num_elems=ne, num_idxs=ni,
                )
                nc.gpsimd.local_scatter(
                    do_t[img][ci][0:ch, :], gd[0:ch, c0 : c0 + ni],
                    io_t[ci][0:ch, 0:ni],
                    channels=ch, num_elems=ne, num_idxs=ni,
                )

            def dfold_ci(img, ci):
                # fold transposed pieces from Gt[cj>=ci] into Gt[ci] -- all
                # block transposes land in ONE psum tile so a single DVE add
                # folds them -- then scatter.
                G = Gt[img]
                nblk = 4 - ci
                c0 = 128 * ci
                tp = ps.tile([128, 512], F32, tag="pbig", name="pbig")
                for j in range(nblk):
                    nc.tensor.matmul(
                        tp[:, 128 * j : 128 * j + 128],
                        G[ci + j][:, c0 : c0 + 128], ident[:],
                        is_transpose=True, skip_group_check=True,
                    )
                if ci > 0:
                    # G4 block transposed into the spare columns of tp
                    g4c = 128 * nblk
                    nc.tensor.matmul(
                        tp[:, g4c : g4c + 128], G[4][:, c0 : c0 + 128], ident[:],
                        is_transpose=True, skip_group_check=True,
                    )
                    nc.vector.tensor_tensor(
                        out=G[ci][:, c0 : c0 + 128 * nblk],
                        in0=G[ci][:, c0 : c0 + 128 * nblk],
                        in1=tp[:, 0 : 128 * nblk], op=AT.add,
                    )
                    nc.vector.tensor_tensor(
                        out=G[ci][:, 512:513], in0=G[ci][:, 512:513],
                        in1=tp[:, g4c : g4c + 1], op=AT.add,
                    )
                else:
                    tp4f = ps.tile([128, 512], F32, tag="pbig", name="pbig")
                    nc.tensor.matmul(
                        tp4f[:, 0:128], G[4][:, c0 : c0 + 128], ident[:],
                        is_transpose=True, skip_group_check=True,
                    )
                    nc.vector.tensor_tensor(
                        out=G[ci][:, 0:512], in0=G[ci][:, 0:512],
                        in1=tp[:, 0:512], op=AT.add,
                    )
                    nc.vector.tensor_tensor(
                        out=G[ci][:, 512:513], in0=G[ci][:, 512:513],
                        in1=tp4f[:, 0:1], op=AT.add,
                    )
                scat(img, ci)

            red_ps = {}

            def red_ci(img, ci):
                # accumulate ones @ de/do into radial-sum rows: bins 0..511 on
                # partition 0, bins 512..725 on partition 32 of one PSUM bank.
                # emission order must be ci = 3, 4, 2, 1, 0 per image.
                ch = 128 if ci < 4 else 16
                c0, ni, b0, ne = SCAT[ci]
                if ci == 3:
                    red_ps[img] = ps1.tile([128, 512], F32, tag="rlo", name="rlo")
                rf = red_ps[img]
                lo_w = max(0, 512 - b0)            # de cols [0, lo_w) -> bins b0..
                hs = lo_w                          # de cols [hs, ne) -> bins >= 512
                for j, dst in enumerate((de_t[img][ci], do_t[img][ci])):
                    if lo_w > 0:
                        st_, sp = (ci == 2 and j == 0), (ci == 0 and j == 1)
                        nc.tensor.matmul(rf[0:1, b0 : b0 + lo_w], onesb[0:ch, :],
                                         dst[0:ch, 0:lo_w], start=st_, stop=sp)
                    st_, sp = (ci == 3 and j == 0), (ci == 0 and j == 1)
                    nc.tensor.matmul(rf[32:33, b0 + hs - 512 : b0 + ne - 512],
                                     onesb[0:ch, :], dst[0:ch, hs:ne],
                                     start=st_, stop=sp, skip_group_check=True)

            def red_fin(img):
                rf = red_ps[img]
                ssq = wpool.tile([33, WB], F32, tag="ssq", bufs=1, name="ssq")
                nc.scalar.activation(ssq[0:1, 0:512], rf[0:1, 0:512], AF.Square)
                nc.scalar.activation(ssq[32:33, 512:726], rf[32:33, 0:214],
                                     AF.Square)
                nc.vector.tensor_tensor(out=ssq[0:1, 0:512], in0=ssq[0:1, 0:512],
                                        in1=ic_t[0:1, 0:512], op=AT.mult)
                nc.vector.tensor_tensor(out=ssq[32:33, 512:726],
                                        in0=ssq[32:33, 512:726],
                                        in1=ic_t[32:33, 512:726], op=AT.mult)
                # q2 partial sums land in p2acc cols 1 (p0) and 1 (p32);
                # the ones-matmul then gives p2 in col 0 and q2 in col 1.
                with nc.allow_low_precision(reason="f32r stores exact f32 bits"):
                    nc.vector.tensor_reduce(
                        p2[img][0:1, 1:2], ssq[0:1, 0:512],
                        axis=mybir.AxisListType.X, op=AT.add
                    )
                    nc.vector.tensor_reduce(
                        p2[img][32:33, 1:2], ssq[32:33, 512:726],
                        axis=mybir.AxisListType.X, op=AT.add
                    )
                pspf = ps.tile([128, 16], F32, tag="psml", bufs=1, name="psml")
                psp = pspf[:, 0:8]
                nc.tensor.matmul(psp[0:1, 0:8], ones[:], p2[img][:], start=True,
                                 stop=True)
                pv = wpool.tile([1, 2], F32, tag="pv", name="pv")
                nc.vector.tensor_copy(pv[0:1, 0:2], psp[0:1, 0:2])
                nc.vector.tensor_tensor(
                    out=lossv[0:1, img : img + 1], in0=pv[0:1, 0:1],
                    in1=pv[0:1, 1:2], op=AT.subtract,
                )

            # ================ emission schedule ================
            # setup constants needed first
            make_identity(nc, ident[:])
            nc.gpsimd.memset(ones32[:], 1.0)
            nc.vector.tensor_copy(ones[:], ones32[:])
            nc.vector.tensor_copy(onesb[:], ones32[:])
            nc.gpsimd.memset(zt[:], 0.0)

            # img0 chunk rounds: image + step-1 trig interleaved
            sc = nc.named_scope("rf_0"); sc.__enter__()
            for k in range(4):
                lo, hi = dma_round(0, k)
                nc.sync.dma_start(ct_t[k][:], ct_p[128 * k : 128 * k + 128, :])
                fold_chunk(0, k, lo, hi)
            lo5_0 = dma_row512(0)
            nc.sync.dma_start(ct512[:], ct_p[512:513, :])
            for k in range(4):
                nc.sync.dma_start(st_t[k][:], st_p[128 * k : 128 * k + 128, :])
            fold_row512(0, lo5_0)
            sc.__exit__(None, None, None)

            # step-2 trig + chunk-4 scatter consts (needed ~mid-step1)
            for k in range(4):
                nc.sync.dma_start(c2_t[k][:], c2_p[128 * k : 128 * k + 128, :])
                nc.sync.dma_start(s2_t[k][:], s2_p[128 * k : 128 * k + 128, :])
            nc.sync.dma_start(c2x_t[:], c2_p[512:513, :])
            nc.sync.dma_start(ie_t[4][:], ie_p[512:640, :])
            nc.sync.dma_start(io_t[4][:], io_p[512:640, :])
            nc.sync.dma_start(mf_t[4][:], mf_p[512:640, :])
            nc.sync.dma_start(wc_t[:], wc_p[:])
            nc.sync.dma_start(ic_t[:], ic_p[:])

            # step 1 img0
            sc = nc.named_scope("s1_0"); sc.__enter__()
            s1_alloc(0)
            for fam in FAMS:
                for m in range(4):
                    s1_group(0, fam, m)
            s1_a512(0, "ee")
            s1_a512(0, "oe")
            sc.__exit__(None, None, None)

            # img1 input + remaining scatter consts; fold img1 (DVE+gpsimd)
            # overlaps step-1 img0 (PE)
            sc = nc.named_scope("rf_1"); sc.__enter__()
            for k in range(4):
                lo, hi = dma_round(1, k)
                fold_chunk(1, k, lo, hi)
            lo5_1 = dma_row512(1)
            for ci in range(4):
                nc.sync.dma_start(ie_t[ci][:], ie_p[128 * ci : 128 * ci + 128, :])
                nc.sync.dma_start(io_t[ci][:], io_p[128 * ci : 128 * ci + 128, :])
                nc.sync.dma_start(mf_t[ci][:], mf_p[128 * ci : 128 * ci + 128, :])
            fold_row512(1, lo5_1)
            sc.__exit__(None, None, None)

            # step 1 img1 (img0 post-processing has nothing to run against yet;
            # both images' step-2 streams are merged below instead)
            sc = nc.named_scope("s1_1"); sc.__enter__()
            s1_alloc(1)
            for fam in FAMS:
                for m in range(4):
                    s1_group(1, fam, m)
            s1_a512(1, "ee")
            s1_a512(1, "oe")
            sc.__exit__(None, None, None)

            s2_alloc(0)
            df_alloc(0)
            s2_alloc(1)
            df_alloc(1)

            # merged step-2: alternate images so each mu's post-processing and
            # PSUM evacuation hides behind the other image's matmuls; the
            # diagonal folds, scatters, and radial reductions stream behind.
            sc = nc.named_scope("s2m"); sc.__enter__()
            s2_u512(0)
            scat(0, 4)
            s2_mu(0, 3)
            s2_u512(1)
            scat(1, 4)
            s2_mu(1, 3)
            dfold_ci(0, 3)
            s2_mu(0, 2)
            dfold_ci(1, 3)
            s2_mu(1, 2)
            red_ci(0, 3)
            red_ci(0, 4)
            dfold_ci(0, 2)
            s2_mu(0, 1)
            dfold_ci(1, 2)
            s2_mu(1, 1)
            red_ci(0, 2)
            dfold_ci(0, 1)
            s2_mu(0, 0, defer_x=True)
            dfold_ci(1, 1)
            s2_x_run(0)
            s2_mu(1, 0, defer_x=True)
            red_ci(0, 1)
            dfold_ci(0, 0)
            dfold_ci(1, 0)
            s2_x_run(1)
            sc.__exit__(None, None, None)

            sc = nc.named_scope("red_1"); sc.__enter__()
            red_ci(0, 0)
            red_fin(0)
            red_ci(1, 3)
            red_ci(1, 4)
            red_ci(1, 2)
            red_ci(1, 1)
            red_ci(1, 0)
            red_fin(1)
            sc.__exit__(None, None, None)

            nc.sync.dma_start(out_p[:], lossv[:])

    nc.compile()
    return nc


def _get_nc():
    if "nc" not in _CACHE:
        _CACHE["nc"] = _build_nc()
    return _CACHE["nc"]


# ---------------------------------------------------------------- entry point
def kernel(prob_cg: np.ndarray) -> np.ndarray:
    import ml_dtypes

    hc = _host_constants()
    nc = _get_nc()
    bf = ml_dtypes.bfloat16
    x = prob_cg[:, 0, :, :].astype(bf)
    B = x.shape[0]
    xlo = np.ascontiguousarray(x[:, 0:NQ, :])
    xhr = np.zeros((B, NQ, H), dtype=bf)
    xhr[:, 1:512] = x[:, 1023:512:-1, :]
    in_maps = []
    for i in range(N_CORES):
        in_maps.append(
            dict(
                xlo=xlo[2 * i : 2 * i + 2],
                xhr=xhr[2 * i : 2 * i + 2],
                ct=hc["ct"], st=hc["st"], c2=hc["c2"], s2=hc["s2"],
                ie=hc["ie"], io=hc["io"], mf=hc["mfull_bf16"],
                wc=hc["wc"], ic=hc["invc"],
            )
        )
    trace = os.environ.get("AT_TRACE", "0") == "1"
    res = run_bass_kernel_spmd(nc, in_maps, core_ids=list(range(N_CORES)), trace=trace)
    if trace and res.exec_time_ns is not None:
        print(f"HW exec time: {res.exec_time_ns} ns")
        if res.profile_json:
            print(f"  profile json: {res.profile_json}")
        if res.per_core_scope_times:
            for kname, v in sorted(res.per_core_scope_times.items()):
                print(f"  scope {kname}: {v}")
    losses = np.concatenate([r["out"].reshape(-1) for r in res.results])
    loss = losses.mean() + (H * H) * (EPS * EPS)
    return np.float32(WA * loss)


# revision 36
# speedup vs baseline: 1.0152x; 1.0152x over previous
"""AnisotropySuppressionLoss on 8 TRN2 NeuronCores (Bass/Tile).

Quadrant-folded real-input 2D DFT in bf16, now folded along BOTH axes before
step 1: the host ships the image as bf16 in two row blocks (rows 0..512, and
rows 1023..513 reversed so the row-fold r -> 1024-r is an aligned tile
add/sub).  Step 1 then contracts over only 513 rows (4 chunks + 1 row)
instead of 1024 -- half the MACs and half the input DMA of the previous
version.

With CE/SO/CE2/SO2 the four step-2 partial sums (cos/sin x even/odd folds):
  Fr(v) = CE+SO, Fr(1024-v) = CE-SO, Fi(v) = CE2-SO2, Fi(1024-v) = CE2+SO2.
With sqrt(2*w_u) baked into the trig matrices:
  SQ := CE^2+SO^2+CE2^2+SO2^2  equals the quadrant fold w_u*(P(v)+P(1024-v)),
  X  := CE*SO - CE2*SO2,  and  Plo^2+Phi^2 = SQ^2/2 + 2*X^2
so the radial-bin input G is just SQ (cols 0/512 halved) and the p2 term
accumulates from SQ and X row-reductions. Radial segment sums use the exact
barrel-free local_scatter; loss_img = sum_w P^2 - sum_k S_k^2/c_k.

Schedule: chunk-granular DMA -> fold -> step-1 pipeline (matmuls start as
soon as the first 128-row chunk lands); step 2 runs v-chunks in descending
order so the diagonal fold + scatters + radial reductions stream out behind
it; image 1's step-1 groups are interleaved between image 0's step-2 groups
to keep the PE dense (HAM stays warm).  Data-parallel: batch 16 -> 2 images
per core; host averages.
"""

import os
import sys

sys.path.insert(0, "/opt/trn_rl_repo")

import numpy as np

import concourse.bass as bass
import concourse.tile as tile
from concourse import bacc, mybir
from concourse.bass_utils import run_bass_kernel_spmd
from concourse.masks import make_identity

F32 = mybir.dt.float32
F32R = mybir.dt.float32r
BF16 = mybir.dt.bfloat16

H = 1024
NQ = 513          # quadrant size (0..512 per axis)
NB = 725          # radial bins 0..724
WB = 728          # bin buffer width
N_CORES = 8
IMGS_PER_CORE = 2
WA = 0.002
EPS = 1e-12
CHUNKS = [(0, 128), (128, 256), (256, 384), (384, 512), (512, 513)]
# per-chunk scatter windows: chunk ci rows have bins in [floor(sqrt(2)*c0), 725)
SCAT = [(0, 514, 0, 726), (128, 386, 181, 546), (256, 258, 362, 364),
        (384, 130, 543, 184), (512, 2, 724, 2)]
FAMS = ("ee", "oe", "eo", "oo")

_CACHE = {}


# ---------------------------------------------------------------- host consts
def _host_constants():
    if "consts" in _CACHE:
        return _CACHE["consts"]
    import ml_dtypes

    cu = np.arange(NQ, dtype=np.float64)
    wu = np.full(NQ, 2.0)
    wu[0] = 1.0
    wu[512] = 1.0
    sqw = np.sqrt(wu) / 32.0          # step-1 col scale: sqrt(w_u), half of 1/H
    s2c = np.sqrt(2.0) / 32.0         # step-2 scale (makes SQ == G directly)

    # step-1 trig with the r -> 1024-r fold baked in: contract r = 0..512.
    rr = np.arange(NQ, dtype=np.float64)
    angf = 2.0 * np.pi * np.outer(rr, cu) / H
    Ct = np.cos(angf) * sqw[None, :]          # [513, 513] rows r, cols v
    St = (-np.sin(angf) * sqw[None, :])[:512]  # [512, 513]; row 0 is zero

    ang = 2.0 * np.pi * np.outer(cu, cu) / H
    C2 = np.cos(ang) * s2c
    C2[512] *= 0.5                    # xe col 512 is 2x true; compensate
    S2 = (np.sin(ang) * s2c)[:512]

    # p2 per-partition weights: cols 2mu = 0.5/w_u, 2mu+1 = 2/w_u
    wc = np.zeros((128, 10), dtype=np.float32)
    for mu in range(5):
        c0, c1 = CHUNKS[mu]
        n = c1 - c0
        wc[:n, 2 * mu] = (0.5 / wu[c0:c1]).astype(np.float32)
        wc[:n, 2 * mu + 1] = (2.0 / wu[c0:c1]).astype(np.float32)

    # radial bin counts exactly as reference._radial_bins (unshifted coords)
    y = np.minimum(np.arange(H), H - np.arange(H))
    yy, xx = np.meshgrid(y, y, indexing="ij")
    dist = np.sqrt((xx.astype(np.float64)) ** 2 + yy.astype(np.float64) ** 2)
    bins_full = np.clip(dist.astype(np.int32), 0, NB - 1)
    counts = np.bincount(bins_full.reshape(-1), minlength=NB).astype(np.float64)
    invc = np.zeros((33, WB), dtype=np.float32)
    invc[0, :NB] = (1.0 / counts).astype(np.float32)
    invc[32, :NB] = invc[0, :NB]

    # per-(chunk,parity) scatter indices: cell (u, j>=u) -> bin floor(sqrt(u^2+j^2));
    # within a row and j-parity, bins are strictly increasing (dr/dj >= 1/sqrt(2))
    # so each local_scatter sees no duplicate indices. -1 = masked.
    NI = 514
    ie = -np.ones((640, NI), dtype=np.int16)
    io = -np.ones((640, NI), dtype=np.int16)
    for ci in range(5):
        c0, c1 = CHUNKS[ci]
        b0 = SCAT[ci][2]
        for p in range(c1 - c0):
            u = c0 + p
            js = np.arange(u, NQ)
            ks = np.floor(np.sqrt(u * u + js.astype(np.float64) ** 2)).astype(np.int16)
            tgt_e = js % 2 == 0
            ie[128 * ci + p, js[tgt_e] - c0] = ks[tgt_e] - b0
            io[128 * ci + p, js[~tgt_e] - c0] = ks[~tgt_e] - b0

    # full-width pre-masks: 0 below diag, 0.5 on diag (doubled by fold),
    # 1 above; chunk 4 keeps its single diagonal cell at weight 1.
    mfull = np.zeros((640, NQ), dtype=np.float32)
    for ci in range(4):
        a = 128 * ci + np.arange(128)
        cols = np.arange(NQ)
        blk = (cols[None, :] > a[:, None]).astype(np.float32)
        blk[np.arange(128), a] = 0.5
        mfull[128 * ci : 128 * ci + 128] = blk
    mfull[512, 512] = 1.0

    bf = ml_dtypes.bfloat16
    _CACHE["consts"] = dict(
        ct=Ct.astype(bf), st=St.astype(bf), c2=C2.astype(bf), s2=S2.astype(bf),
        wc=wc, invc=invc, ie=ie, io=io,
        mfull_bf16=mfull.astype(bf), counts=counts,
    )
    return _CACHE["consts"]


# ---------------------------------------------------------------- device build
def _build_nc():
    nc = bacc.Bacc("TRN2", target_bir_lowering=False, debug=False)
    xlo_p = nc.declare_dram_parameter("xlo", [IMGS_PER_CORE, NQ, H], BF16, isOutput=False)
    xhr_p = nc.declare_dram_parameter("xhr", [IMGS_PER_CORE, NQ, H], BF16, isOutput=False)
    ct_p = nc.declare_dram_parameter("ct", [NQ, NQ], BF16, isOutput=False)
    st_p = nc.declare_dram_parameter("st", [512, NQ], BF16, isOutput=False)
    c2_p = nc.declare_dram_parameter("c2", [NQ, NQ], BF16, isOutput=False)
    s2_p = nc.declare_dram_parameter("s2", [512, NQ], BF16, isOutput=False)
    ie_p = nc.declare_dram_parameter("ie", [640, 514], mybir.dt.int16, isOutput=False)
    io_p = nc.declare_dram_parameter("io", [640, 514], mybir.dt.int16, isOutput=False)
    mf_p = nc.declare_dram_parameter("mf", [640, NQ], BF16, isOutput=False)
    wc_p = nc.declare_dram_parameter("wc", [128, 10], F32, isOutput=False)
    ic_p = nc.declare_dram_parameter("ic", [33, WB], F32, isOutput=False)
    out_p = nc.declare_dram_parameter("out", [1, IMGS_PER_CORE], F32, isOutput=True)

    AT = mybir.AluOpType
    AF = mybir.ActivationFunctionType

    with tile.TileContext(nc) as tc:
        with (
            tc.tile_pool(name="const", bufs=1) as cpool,
            tc.tile_pool(name="xin", bufs=2) as xpool,
            tc.tile_pool(name="fold", bufs=2) as fpool,
            tc.tile_pool(name="amat", bufs=2) as apool,
            tc.tile_pool(name="quad", bufs=2) as qpool,
            tc.tile_pool(name="dedo", bufs=2) as dpool,
            tc.tile_pool(name="work", bufs=2) as wpool,
            tc.tile_pool(name="ps", bufs=2, space="PSUM") as ps,
            tc.tile_pool(name="ps1", bufs=1, space="PSUM") as ps1,
        ):
            # ---------------- constant tiles
            ct_t = [cpool.tile([128, NQ], BF16, tag=f"ct{k}", name=f"ct{k}") for k in range(4)]
            st_t = [cpool.tile([128, NQ], BF16, tag=f"st{k}", name=f"st{k}") for k in range(4)]
            ct512 = cpool.tile([1, NQ], BF16, tag="ct512", name="ct512")
            c2_t = [cpool.tile([128, NQ], BF16, tag=f"c2{k}", name=f"c2{k}") for k in range(4)]
            s2_t = [cpool.tile([128, NQ], BF16, tag=f"s2{k}", name=f"s2{k}") for k in range(4)]
            c2x_t = cpool.tile([1, NQ], BF16, tag="c2x", name="c2x")
            ie_t = [cpool.tile([128, 514], mybir.dt.int16, tag=f"ie{ci}", name=f"ie{ci}")
                    for ci in range(5)]
            io_t = [cpool.tile([128, 514], mybir.dt.int16, tag=f"io{ci}", name=f"io{ci}")
                    for ci in range(5)]
            mf_t = [cpool.tile([128, NQ], BF16, tag=f"mf{ci}", name=f"mf{ci}") for ci in range(5)]
            wc_t = cpool.tile([128, 10], F32, tag="wc", name="wc")
            ic_t = cpool.tile([33, WB], F32, tag="ic", name="ic")
            ident = cpool.tile([128, 128], F32, tag="ident", name="ident")
            ones32 = cpool.tile([128, 1], F32, tag="ones32", name="ones32")
            ones = cpool.tile([128, 1], F32R, tag="ones", name="ones")
            onesb = cpool.tile([128, 1], BF16, tag="onesb", name="onesb")
            zt = cpool.tile([128, WB], F32, tag="zt", name="zt")
            lossv = cpool.tile([1, IMGS_PER_CORE], F32, tag="lossv", name="lossv")

            # ---------------- DMA helpers (emission order == queue order)
            def dma_round(img, k):
                lo = xpool.tile([128, H], BF16, tag="lo", bufs=3, name=f"lo{k}")
                hi = xpool.tile([128, H], BF16, tag="hi", bufs=3, name=f"hi{k}")
                nc.sync.dma_start(lo[:], xlo_p[img, 128 * k : 128 * k + 128, :])
                nc.sync.dma_start(hi[:], xhr_p[img, 128 * k : 128 * k + 128, :])
                return lo, hi

            def dma_row512(img):
                lo5 = xpool.tile([1, H], BF16, tag="lo5", name="lo5")
                nc.sync.dma_start(lo5[:], xlo_p[img, 512:513, :])
                return lo5

            # ---------------- fold: r-fold (tile add) then c-fold (in-row)
            fq = {}    # fq[img][fam][k] fold tiles; fq5[img][fam] row-512
            fq5 = {}

            def fold_chunk(img, k, lo, hi):
                xE = fpool.tile([128, H], BF16, tag="xE", name=f"xE{k}")
                xO = fpool.tile([128, H], BF16, tag="xO", name=f"xO{k}")
                nc.vector.tensor_tensor(out=xE[:], in0=lo[:], in1=hi[:], op=AT.add)
                nc.vector.tensor_tensor(out=xO[:], in0=lo[:], in1=hi[:], op=AT.subtract)
                out = {}
                for fam, src in (("ee", xE), ("eo", xE), ("oe", xO), ("oo", xO)):
                    f = fpool.tile([128, NQ], BF16, tag=f"f{fam}{k}", name=f"f{fam}{k}")
                    even = fam in ("ee", "oe")
                    eng = nc.vector if even else nc.gpsimd
                    eng.tensor_tensor(
                        out=f[:, 1:513], in0=src[:, 1:513],
                        in1=src[:, 1023:511:-1],
                        op=AT.add if even else AT.subtract,
                    )
                    if even:
                        nc.scalar.activation(f[:, 0:1], src[:, 0:1], AF.Copy)
                    else:
                        nc.gpsimd.memset(f[:, 0:1], 0.0)
                    out[fam] = f
                fq.setdefault(img, {f: [None] * 4 for f in FAMS})
                for fam in FAMS:
                    fq[img][fam][k] = out[fam]

            def fold_row512(img, lo5):
                # r = 512: xE row = x[512] (xhr row 512 is zero), xO row = 0.
                # only cos families need it (sin row 512 is zero).
                d = {}
                for fam, op in (("ee", AT.add), ("eo", AT.subtract)):
                    f = fpool.tile([1, NQ], BF16, tag=f"f5{fam}", name=f"f5{fam}")
                    nc.vector.tensor_tensor(
                        out=f[0:1, 1:513], in0=lo5[0:1, 1:513],
                        in1=lo5[0:1, 1023:511:-1], op=op,
                    )
                    if fam == "ee":
                        nc.scalar.activation(f[0:1, 0:1], lo5[0:1, 0:1], AF.Copy)
                    else:
                        nc.scalar.activation(f[0:1, 0:1], zt[0:1, 0:1], AF.Copy)
                    d[fam] = f
                fq5[img] = d

            # ---------------- step 1: A = fold.T @ trig  (contract 513 rows)
            A_t = {}   # A_t[img][fam][m] [128,512] bf16
            Acol = {}  # Acol[img][fam] [128,4]
            A512 = {}  # A512[img][fam] [1,NQ]

            def s1_alloc(img):
                A_t[img] = {fam: [apool.tile([128, 512], BF16, tag=f"A{fam}{m}",
                                             name=f"A{fam}{m}") for m in range(4)]
                            for fam in FAMS}
                Acol[img] = {fam: apool.tile([128, 4], BF16, tag=f"Ac{fam}", name=f"Ac{fam}")
                             for fam in ("ee", "eo")}
                A512[img] = {fam: apool.tile([1, NQ], BF16, tag=f"A5{fam}", name=f"A5{fam}")
                             for fam in ("ee", "oe")}

            def s1_group(img, fam, m):
                # one (fam, u-chunk) output: psA [128,512] over 4(+1) r-chunks
                cos_fam = fam in ("ee", "eo")
                rhsM = ct_t if cos_fam else st_t
                xf = fq[img][fam]
                psA = ps.tile([128, 512], F32, tag="pbig", name="pbig")
                if cos_fam:
                    psAc = ps.tile([128, 16], F32, tag="psml", bufs=1, name="psml")
                for k in range(4):
                    st_, sp = (k == 0), (k == 3 and not cos_fam)
                    lhs = xf[k][:, 128 * m : 128 * m + 128]
                    nc.tensor.matmul(psA[:], lhs, rhsM[k][:, 0:512], start=st_, stop=sp)
                    if cos_fam:
                        nc.tensor.matmul(psAc[:, 0:1], lhs, rhsM[k][:, 512:513],
                                         start=st_, stop=False)
                if cos_fam:
                    lhs5 = fq5[img][fam][0:1, 128 * m : 128 * m + 128]
                    nc.tensor.matmul(psA[:], lhs5, ct512[0:1, 0:512],
                                     start=False, stop=True, skip_group_check=True)
                    nc.tensor.matmul(psAc[:, 0:1], lhs5, ct512[0:1, 512:513],
                                     start=False, stop=True, skip_group_check=True)
                nc.scalar.activation(A_t[img][fam][m][:], psA[:], AF.Copy)
                if cos_fam:
                    nc.scalar.activation(Acol[img][fam][:, m : m + 1],
                                         psAc[:, 0:1], AF.Copy)

            def s1_a512(img, fam):
                # u = 512 row of A for fams ee (cos) / oe (sin): M=1 matmuls
                cos_fam = fam == "ee"
                rhsM = ct_t if cos_fam else st_t
                xf = fq[img]["ee" if cos_fam else "oe"]
                psRf = ps.tile([128, 512], F32, tag="pbig", name="pbig")
                psR = psRf[0:1, :]
                psRcf = ps.tile([128, 16], F32, tag="psml", bufs=1, name="psml")
                psRc = psRcf[0:1, 0:1]
                for k in range(4):
                    st_, sp = (k == 0), (k == 3 and not cos_fam)
                    lhs = xf[k][:, 512:513]
                    nc.tensor.matmul(psR, lhs, rhsM[k][:, 0:512], start=st_, stop=sp)
                    if cos_fam:
                        nc.tensor.matmul(psRc, lhs, rhsM[k][:, 512:513],
                                         start=st_, stop=False)
                if cos_fam:
                    lhs5 = fq5[img]["ee"][0:1, 512:513]
                    nc.tensor.matmul(psR, lhs5, ct512[0:1, 0:512],
                                     start=False, stop=True, skip_group_check=True)
                    nc.tensor.matmul(psRc, lhs5, ct512[0:1, 512:513],
                                     start=False, stop=True, skip_group_check=True)
                    nc.scalar.activation(A512[img]["ee"][0:1, 0:512], psR, AF.Copy)
                    nc.scalar.activation(A512[img]["ee"][0:1, 512:513], psRc, AF.Copy)
                else:
                    nc.scalar.activation(A512[img]["oe"][0:1, 0:512], psR, AF.Copy)
                    # sin col 512 is exactly zero
                    nc.scalar.activation(A512[img]["oe"][0:1, 512:513], zt[0:1, 0:1],
                                         AF.Copy)

            # ---------------- step 2 state
            Gt = {}
            p2 = {}

            def s2_alloc(img):
                Gt[img] = [qpool.tile([128, 516], F32, tag=f"g{ci}", name=f"g{ci}")
                           for ci in range(5)]
                p2[img] = qpool.tile([128, 8], F32R, tag="p2acc", name="p2acc")
                nc.vector.tensor_copy(p2[img][:, 0:8], zt[:, 0:8])

            s2x = {}

            def s2_mu(img, mu, defer_x=False):
                p2acc = p2[img]
                At = A_t[img]
                M = 128
                u0 = 128 * mu
                if img == 0:
                    psCE = ps1.tile([128, 512], F32, tag="p2ce", name="p2ce")
                    psCE2 = ps1.tile([128, 512], F32, tag="p2ce2", name="p2ce2")
                else:
                    psCE = ps.tile([128, 512], F32, tag="pbig", name="pbig")
                    psCE2 = ps.tile([128, 512], F32, tag="pbig", name="pbig")
                psSO = ps1.tile([128, 512], F32, tag="p2so", name="p2so")
                psSO2 = ps1.tile([128, 512], F32, tag="p2so2", name="p2so2")
                pscc = ps.tile([128, 16], F32, tag="psml", bufs=1, name="psml")
                psc = pscc[:, 0:8]
                psc2 = pscc[:, 8:16]
                for k in range(4):
                    st_ = (k == 0)
                    lee = At["ee"][k][:, u0 : u0 + M]
                    loe = At["oe"][k][:, u0 : u0 + M]
                    nc.tensor.matmul(psCE[0:M], lee, c2_t[k][:, 0:512],
                                     start=st_, stop=False)
                    nc.tensor.matmul(psc[0:M, 0:1], lee, c2_t[k][:, 512:513],
                                     start=st_, stop=False)
                    nc.tensor.matmul(psCE2[0:M], loe, c2_t[k][:, 0:512],
                                     start=st_, stop=False)
                    nc.tensor.matmul(psc2[0:M, 0:1], loe, c2_t[k][:, 512:513],
                                     start=st_, stop=False, skip_group_check=True)
                xee = A512[img]["ee"][0:1, u0 : u0 + M]
                xoe = A512[img]["oe"][0:1, u0 : u0 + M]
                nc.tensor.matmul(psCE[0:M], xee, c2x_t[0:1, 0:512],
                                 start=False, stop=True, skip_group_check=True)
                nc.tensor.matmul(psc[0:M, 0:1], xee, c2x_t[0:1, 512:513],
                                 start=False, stop=True, skip_group_check=True)
                nc.tensor.matmul(psCE2[0:M], xoe, c2x_t[0:1, 0:512],
                                 start=False, stop=True, skip_group_check=True)
                nc.tensor.matmul(psc2[0:M, 0:1], xoe, c2x_t[0:1, 512:513],
                                 start=False, stop=True, skip_group_check=True)
                for k in range(4):
                    st_ = (k == 0)
                    loo = At["oo"][k][:, u0 : u0 + M]
                    leo = At["eo"][k][:, u0 : u0 + M]
                    nc.tensor.matmul(psSO[0:M], loo, s2_t[k][:, 0:512],
                                     start=st_, stop=(k == 3))
                    nc.tensor.matmul(psSO2[0:M], leo, s2_t[k][:, 0:512],
                                     start=st_, stop=(k == 3))

                # square-during-evacuation on scalar (PSUM f32 -> bf16 squares);
                # the X cross-products read PSUM directly on vector.
                sqa = wpool.tile([128, 512], BF16, tag="sqa", name="sqa")
                sqb = wpool.tile([128, 512], BF16, tag="sqb", name="sqb")
                sqc = wpool.tile([128, 512], BF16, tag="sqc", name="sqc")
                sqd = wpool.tile([128, 512], BF16, tag="sqd", name="sqd")
                sqe = wpool.tile([128, 2], F32, tag="sqe", name="sqe")
                def emit_x_products():
                    q1 = wpool.tile([128, 512], BF16, tag="q1", name="q1")
                    q2w = wpool.tile([128, 512], BF16, tag="q2w", name="q2w")
                    sos = wpool.tile([128, 512], BF16, tag="sos", name="sos")
                    so2s = wpool.tile([128, 512], BF16, tag="so2s", name="so2s")
                    nc.vector.tensor_copy(sos[0:M], psSO[0:M])
                    nc.vector.tensor_copy(so2s[0:M], psSO2[0:M])
                    nc.vector.tensor_tensor(out=q1[0:M], in0=sos[0:M],
                                            in1=psCE[0:M], op=AT.mult)
                    nc.vector.tensor_tensor(out=q2w[0:M], in0=so2s[0:M],
                                            in1=psCE2[0:M], op=AT.mult)
                    return q1, q2w

                def emit_x_tail(q1, q2w):
                    xp = wpool.tile([128, 512], BF16, tag="xp", bufs=1, name="xp")
                    nc.vector.tensor_tensor(out=xp[0:M], in0=q1[0:M],
                                            in1=q2w[0:M], op=AT.subtract)
                    junk2 = wpool.tile([128, 513], BF16, tag="junk", bufs=1,
                                       name="junk")
                    xrs = wpool.tile([128, 1], F32, tag="xrs", name="xrs")
                    nc.scalar.activation(junk2[0:M, 0:512], xp[0:M], AF.Square,
                                         accum_out=xrs[0:M])
                    nc.vector.scalar_tensor_tensor(
                        p2acc[0:M, 0:1], xrs[0:M],
                        wc_t[0:M, 2 * mu + 1 : 2 * mu + 2],
                        p2acc[0:M, 0:1], op0=AT.mult, op1=AT.add,
                    )

                if defer_x:
                    s2x[img] = (emit_x_products, emit_x_tail)
                else:
                    _q1, _q2w = emit_x_products()
                nc.scalar.activation(sqa[0:M], psCE[0:M], AF.Square)
                nc.scalar.activation(sqb[0:M], psSO[0:M], AF.Square)
                nc.scalar.activation(sqc[0:M], psCE2[0:M], AF.Square)
                nc.scalar.activation(sqd[0:M], psSO2[0:M], AF.Square)
                nc.scalar.activation(sqe[0:M, 0:2], pscc[0:M, 0:16:8], AF.Square,
                                     scale=float(np.sqrt(0.5)))
                s1t = wpool.tile([128, 512], BF16, tag="s1t", bufs=1, name="s1t")
                s2w = wpool.tile([128, 512], BF16, tag="s2w", bufs=1, name="s2w")
                nc.vector.tensor_tensor(out=s1t[0:M], in0=sqa[0:M], in1=sqb[0:M],
                                        op=AT.add)
                nc.vector.tensor_tensor(out=s2w[0:M], in0=sqc[0:M], in1=sqd[0:M],
                                        op=AT.add)
                G = Gt[img][mu]
                nc.vector.tensor_tensor(out=G[0:M, 0:512], in0=s1t[0:M],
                                        in1=s2w[0:M], op=AT.add)
                nc.vector.tensor_tensor(out=G[0:M, 512:513], in0=sqe[0:M, 0:1],
                                        in1=sqe[0:M, 1:2], op=AT.add)
                nc.vector.tensor_scalar_mul(G[0:M, 0:1], G[0:M, 0:1], 0.5)
                if mu == 0:
                    nc.vector.tensor_scalar_mul(G[0:1, 0:1], G[0:1, 0:1], 0.0)
                junk = wpool.tile([128, 513], BF16, tag="junk", bufs=1, name="junk")
                srs = wpool.tile([128, 1], F32, tag="srs", name="srs")
                crs = wpool.tile([128, 1], F32, tag="crs", name="crs")
                nc.scalar.activation(junk[0:M, 0:513], G[0:M, 0:513],
                                     AF.Square, accum_out=srs[0:M])
                nc.scalar.activation(junk[0:M, 0:2], G[0:M, 0:513:512],
                                     AF.Square, accum_out=crs[0:M])
                nc.vector.scalar_tensor_tensor(
                    p2acc[0:M, 0:1], srs[0:M], wc_t[0:M, 2 * mu : 2 * mu + 1],
                    p2acc[0:M, 0:1], op0=AT.mult, op1=AT.add,
                )
                nc.vector.scalar_tensor_tensor(
                    p2acc[0:M, 0:1], crs[0:M], wc_t[0:M, 2 * mu : 2 * mu + 1],
                    p2acc[0:M, 0:1], op0=AT.mult, op1=AT.add,
                )
                if not defer_x:
                    emit_x_tail(_q1, _q2w)

            def s2_x_run(img):
                prod, tail = s2x.pop(img)
                tail(*prod())

            def s2_u512(img):
                # u = 512 (Nyquist) row: SO and CE2 are exactly 0, X = 0
                p2acc = p2[img]
                psCE = ps1.tile([128, 512], F32, tag="p2ce", name="p2ce")
                psSO2 = ps1.tile([128, 512], F32, tag="p2so2", name="p2so2")
                pscf = ps.tile([128, 16], F32, tag="psml", bufs=1, name="psml")
                psc = pscf[:, 0:8]
                for k in range(4):
                    st_ = (k == 0)
                    nc.tensor.matmul(psCE[0:1], Acol[img]["ee"][:, k : k + 1],
                                     c2_t[k][:, 0:512], start=st_, stop=False)
                    nc.tensor.matmul(psc[0:1, 0:1], Acol[img]["ee"][:, k : k + 1],
                                     c2_t[k][:, 512:513], start=st_, stop=False)
                    nc.tensor.matmul(psSO2[0:1], Acol[img]["eo"][:, k : k + 1],
                                     s2_t[k][:, 0:512], start=st_, stop=(k == 3))
                nc.tensor.matmul(psCE[0:1], A512[img]["ee"][0:1, 512:513],
                                 c2x_t[0:1, 0:512],
                                 start=False, stop=True, skip_group_check=True)
                nc.tensor.matmul(psc[0:1, 0:1], A512[img]["ee"][0:1, 512:513],
                                 c2x_t[0:1, 512:513],
                                 start=False, stop=True, skip_group_check=True)
                sqe = wpool.tile([128, 2], F32, tag="sqe", name="sqe")
                sqa = wpool.tile([128, 512], BF16, tag="sqa", name="sqa")
                sqd = wpool.tile([128, 512], BF16, tag="sqd", name="sqd")
                nc.scalar.activation(sqa[0:1], psCE[0:1], AF.Square)
                nc.scalar.activation(sqd[0:1], psSO2[0:1], AF.Square)
                nc.scalar.activation(sqe[0:1, 0:1], psc[0:1, 0:1], AF.Square,
                                     scale=float(np.sqrt(0.5)))
                G4 = Gt[img][4]
                nc.vector.tensor_tensor(out=G4[0:1, 0:512], in0=sqa[0:1],
                                        in1=sqd[0:1], op=AT.add)
                nc.vector.tensor_copy(G4[0:1, 512:513], sqe[0:1, 0:1])
                nc.vector.tensor_scalar_mul(G4[0:1, 0:1], G4[0:1, 0:1], 0.5)
                junk = wpool.tile([128, 513], BF16, tag="junk", bufs=1, name="junk")
                srs = wpool.tile([128, 1], F32, tag="srs", name="srs")
                crs = wpool.tile([128, 1], F32, tag="crs", name="crs")
                crs2 = wpool.tile([128, 1], F32, tag="crs2", name="crs2")
                nc.scalar.activation(junk[0:1, 0:513], G4[0:1, 0:513],
                                     AF.Square, accum_out=srs[0:1])
                nc.scalar.activation(junk[0:1, 0:1], G4[0:1, 0:1],
                                     AF.Square, accum_out=crs[0:1])
                nc.scalar.activation(junk[0:1, 1:2], G4[0:1, 512:513],
                                     AF.Square, accum_out=crs2[0:1])
                for acc in (srs, crs, crs2):
                    nc.vector.scalar_tensor_tensor(
                        p2acc[0:1, 0:1], acc[0:1], wc_t[0:1, 8:9],
                        p2acc[0:1, 0:1], op0=AT.mult, op1=AT.add,
                    )

            # ---------------- diagonal fold + scatter + radial reduce
            de_t = {}
            do_t = {}

            def df_alloc(img):
                de_t[img] = [dpool.tile([128 if ci < 4 else 16, SCAT[ci][3]], BF16,
                                        tag=f"de{ci}", name=f"de{ci}")
                             for ci in range(5)]
                do_t[img] = [dpool.tile([128 if ci < 4 else 16, SCAT[ci][3]], BF16,
                                        tag=f"do{ci}", name=f"do{ci}")
                             for ci in range(5)]

            def scat(img, ci):
                ch = 128 if ci < 4 else 16
                c0, ni, b0, ne = SCAT[ci]
                gd = wpool.tile([128, 514], BF16, tag=f"gd{ci}", bufs=1,
                                name=f"gd{ci}")
                nc.vector.tensor_tensor(
                    out=gd[:, c0:NQ], in0=Gt[img][ci][:, c0:NQ],
                    in1=mf_t[ci][:, c0:NQ], op=AT.mult,
                )
                nc.gpsimd.local_scatter(
                    de_t[img][ci][0:ch, :], gd[0:ch, c0 : c0 + ni],
                    ie_t[ci][0:ch, 0:ni],
                    channels=ch, num_elems=ne, num_idxs=ni,
                )
                nc.gpsimd.local_scatter(
                    do_t[img][ci][0:ch, :], gd[0:ch, c0 : c0 + ni],
                    io_t[ci][0:ch, 0:ni],
                    channels=ch, num_elems=ne, num_idxs=ni,
                )

            def dfold_ci(img, ci):
                # fold transposed pieces from Gt[cj>=ci] into Gt[ci] -- all
                # block transposes land in ONE psum tile so a single DVE add
                # folds them -- then scatter.
                G = Gt[img]
                nblk = 4 - ci
                c0 = 128 * ci
                tp = ps.tile([128, 512], F32, tag="pbig", name="pbig")
                for j in range(nblk):
                    nc.tensor.matmul(
                        tp[:, 128 * j : 128 * j + 128],
                        G[ci + j][:, c0 : c0 + 128], ident[:],
                        is_transpose=True, skip_group_check=True,
                    )
                if ci > 0:
                    # G4 block transposed into the spare columns of tp
                    g4c = 128 * nblk
                    nc.tensor.matmul(
                        tp[:, g4c : g4c + 128], G[4][:, c0 : c0 + 128], ident[:],
                        is_transpose=True, skip_group_check=True,
                    )
                    nc.vector.tensor_tensor(
                        out=G[ci][:, c0 : c0 + 128 * nblk],
                        in0=G[ci][:, c0 : c0 + 128 * nblk],
                        in1=tp[:, 0 : 128 * nblk], op=AT.add,
                    )
                    nc.vector.tensor_tensor(
                        out=G[ci][:, 512:513], in0=G[ci][:, 512:513],
                        in1=tp[:, g4c : g4c + 1], op=AT.add,
                    )
                else:
                    tp4f = ps.tile([128, 512], F32, tag="pbig", name="pbig")
                    nc.tensor.matmul(
                        tp4f[:, 0:128], G[4][:, c0 : c0 + 128], ident[:],
                        is_transpose=True, skip_group_check=True,
                    )
                    nc.vector.tensor_tensor(
                        out=G[ci][:, 0:512], in0=G[ci][:, 0:512],
                        in1=tp[:, 0:512], op=AT.add,
                    )
                    nc.vector.tensor_tensor(
                        out=G[ci][:, 512:513], in0=G[ci][:, 512:513],
                        in1=tp4f[:, 0:1], op=AT.add,
                    )
                scat(img, ci)

            red_ps = {}

            def red_ci(img, ci):
                # accumulate ones @ de/do into radial-sum rows: bins 0..511 on
                # partition 0, bins 512..725 on partition 32 of one PSUM bank.
                # emission order must be ci = 3, 4, 2, 1, 0 per image.
                ch = 128 if ci < 4 else 16
                c0, ni, b0, ne = SCAT[ci]
                if ci == 3:
                    red_ps[img] = ps1.tile([128, 512], F32, tag="rlo", name="rlo")
                rf = red_ps[img]
                lo_w = max(0, 512 - b0)            # de cols [0, lo_w) -> bins b0..
                hs = lo_w                          # de cols [hs, ne) -> bins >= 512
                for j, dst in enumerate((de_t[img][ci], do_t[img][ci])):
                    if lo_w > 0:
                        st_, sp = (ci == 2 and j == 0), (ci == 0 and j == 1)
                        nc.tensor.matmul(rf[0:1, b0 : b0 + lo_w], onesb[0:ch, :],
                                         dst[0:ch, 0:lo_w], start=st_, stop=sp)
                    st_, sp = (ci == 3 and j == 0), (ci == 0 and j == 1)
                    nc.tensor.matmul(rf[32:33, b0 + hs - 512 : b0 + ne - 512],
                                     onesb[0:ch, :], dst[0:ch, hs:ne],
                                     start=st_, stop=sp, skip_group_check=True)

            def red_fin(img):
                rf = red_ps[img]
                ssq = wpool.tile([33, WB], F32, tag="ssq", bufs=1, name="ssq")
                nc.scalar.activation(ssq[0:1, 0:512], rf[0:1, 0:512], AF.Square)
                nc.scalar.activation(ssq[32:33, 512:726], rf[32:33, 0:214],
                                     AF.Square)
                nc.vector.tensor_tensor(out=ssq[0:1, 0:512], in0=ssq[0:1, 0:512],
                                        in1=ic_t[0:1, 0:512], op=AT.mult)
                nc.vector.tensor_tensor(out=ssq[32:33, 512:726],
                                        in0=ssq[32:33, 512:726],
                                        in1=ic_t[32:33, 512:726], op=AT.mult)
                # q2 partial sums land in p2acc cols 1 (p0) and 1 (p32);
                # the ones-matmul then gives p2 in col 0 and q2 in col 1.
                with nc.allow_low_precision(reason="f32r stores exact f32 bits"):
                    nc.vector.tensor_reduce(
                        p2[img][0:1, 1:2], ssq[0:1, 0:512],
                        axis=mybir.AxisListType.X, op=AT.add
                    )
                    nc.vector.tensor_reduce(
                        p2[img][32:33, 1:2], ssq[32:33, 512:726],
                        axis=mybir.AxisListType.X, op=AT.add
                    )
                pspf = ps.tile([128, 16], F32, tag="psml", bufs=1, name="psml")
                psp = pspf[:, 0:8]
                nc.tensor.matmul(psp[0:1, 0:8], ones[:], p2[img][:], start=True,
                                 stop=True)
                pv = wpool.tile([1, 2], F32, tag="pv", name="pv")
                nc.vector.tensor_copy(pv[0:1, 0:2], psp[0:1, 0:2])
                nc.vector.tensor_tensor(
                    out=lossv[0:1, img : img + 1], in0=pv[0:1, 0:1],
                    in1=pv[0:1, 1:2], op=AT.subtract,
                )

            # ================ emission schedule ================
            # setup constants needed first
            make_identity(nc, ident[:])
            nc.gpsimd.memset(ones32[:], 1.0)
            nc.vector.tensor_copy(ones[:], ones32[:])
            nc.vector.tensor_copy(onesb[:], ones32[:])
            nc.gpsimd.memset(zt[:], 0.0)

            # img0 chunk rounds: image + step-1 trig interleaved
            sc = nc.named_scope("rf_0"); sc.__enter__()
            for k in range(4):
                lo, hi = dma_round(0, k)
                nc.sync.dma_start(ct_t[k][:], ct_p[128 * k : 128 * k + 128, :])
                fold_chunk(0, k, lo, hi)
            lo5_0 = dma_row512(0)
            nc.sync.dma_start(ct512[:], ct_p[512:513, :])
            for k in range(4):
                nc.sync.dma_start(st_t[k][:], st_p[128 * k : 128 * k + 128, :])
            fold_row512(0, lo5_0)
            sc.__exit__(None, None, None)

            # step-2 trig + chunk-4 scatter consts (needed ~mid-step1)
            for k in range(4):
                nc.sync.dma_start(c2_t[k][:], c2_p[128 * k : 128 * k + 128, :])
                nc.sync.dma_start(s2_t[k][:], s2_p[128 * k : 128 * k + 128, :])
            nc.sync.dma_start(c2x_t[:], c2_p[512:513, :])
            nc.sync.dma_start(ie_t[4][:], ie_p[512:640, :])
            nc.sync.dma_start(io_t[4][:], io_p[512:640, :])
            nc.sync.dma_start(mf_t[4][:], mf_p[512:640, :])
            nc.sync.dma_start(wc_t[:], wc_p[:])
            nc.sync.dma_start(ic_t[:], ic_p[:])

            # step 1 img0
            sc = nc.named_scope("s1_0"); sc.__enter__()
            s1_alloc(0)
            for fam in FAMS:
                for m in range(4):
                    s1_group(0, fam, m)
            s1_a512(0, "ee")
            s1_a512(0, "oe")
            sc.__exit__(None, None, None)

            # img1 input + remaining scatter consts; fold img1 (DVE+gpsimd)
            # overlaps step-1 img0 (PE)
            sc = nc.named_scope("rf_1"); sc.__enter__()
            for k in range(4):
                lo, hi = dma_round(1, k)
                fold_chunk(1, k, lo, hi)
            lo5_1 = dma_row512(1)
            for ci in range(4):
                nc.sync.dma_start(ie_t[ci][:], ie_p[128 * ci : 128 * ci + 128, :])
                nc.sync.dma_start(io_t[ci][:], io_p[128 * ci : 128 * ci + 128, :])
                nc.sync.dma_start(mf_t[ci][:], mf_p[128 * ci : 128 * ci + 128, :])
            fold_row512(1, lo5_1)
            sc.__exit__(None, None, None)

            # step 1 img1 (img0 post-processing has nothing to run against yet;
            # both images' step-2 streams are merged below instead)
            sc = nc.named_scope("s1_1"); sc.__enter__()
            s1_alloc(1)
            for fam in FAMS:
                for m in range(4):
                    s1_group(1, fam, m)
            s1_a512(1, "ee")
            s1_a512(1, "oe")
            sc.__exit__(None, None, None)

            s2_alloc(0)
            df_alloc(0)
            s2_alloc(1)
            df_alloc(1)

            # merged step-2: alternate images so each mu's post-processing and
            # PSUM evacuation hides behind the other image's matmuls; the
            # diagonal folds, scatters, and radial reductions stream behind.
            sc = nc.named_scope("s2m"); sc.__enter__()
            s2_u512(0)
            scat(0, 4)
            s2_mu(0, 3)
            s2_u512(1)
            scat(1, 4)
            s2_mu(1, 3)
            dfold_ci(0, 3)
            s2_mu(0, 2)
            dfold_ci(1, 3)
            s2_mu(1, 2)
            red_ci(0, 3)
            red_ci(0, 4)
            dfold_ci(0, 2)
            s2_mu(0, 1)
            dfold_ci(1, 2)
            s2_mu(1, 1)
            red_ci(0, 2)
            dfold_ci(0, 1)
            s2_mu(0, 0, defer_x=True)
            dfold_ci(1, 1)
            s2_mu(1, 0, defer_x=True)
            red_ci(0, 1)
            dfold_ci(0, 0)
            dfold_ci(1, 0)
            s2_x_run(0)
            s2_x_run(1)
            sc.__exit__(None, None, None)

            sc = nc.named_scope("red_1"); sc.__enter__()
            red_ci(0, 0)
            red_fin(0)
            red_ci(1, 3)
            red_ci(1, 4)
            red_ci(1, 2)
            red_ci(1, 1)
            red_ci(1, 0)
            red_fin(1)
            sc.__exit__(None, None, None)

            nc.sync.dma_start(out_p[:], lossv[:])

    nc.compile()
    return nc


def _get_nc():
    if "nc" not in _CACHE:
        _CACHE["nc"] = _build_nc()
    return _CACHE["nc"]


# ---------------------------------------------------------------- entry point
def kernel(prob_cg: np.ndarray) -> np.ndarray:
    import ml_dtypes

    hc = _host_constants()
    nc = _get_nc()
    bf = ml_dtypes.bfloat16
    x = prob_cg[:, 0, :, :].astype(bf)
    B = x.shape[0]
    xlo = np.ascontiguousarray(x[:, 0:NQ, :])
    xhr = np.zeros((B, NQ, H), dtype=bf)
    xhr[:, 1:512] = x[:, 1023:512:-1, :]
    in_maps = []
    for i in range(N_CORES):
        in_maps.append(
            dict(
                xlo=xlo[2 * i : 2 * i + 2],
                xhr=xhr[2 * i : 2 * i + 2],
                ct=hc["ct"], st=hc["st"], c2=hc["c2"], s2=hc["s2"],
                ie=hc["ie"], io=hc["io"], mf=hc["mfull_bf16"],
                wc=hc["wc"], ic=hc["invc"],
            )
        )
    trace = os.environ.get("AT_TRACE", "0") == "1"
    res = run_bass_kernel_spmd(nc, in_maps, core_ids=list(range(N_CORES)), trace=trace)
    if trace and res.exec_time_ns is not None:
        print(f"HW exec time: {res.exec_time_ns} ns")
        if res.profile_json:
            print(f"  profile json: {res.profile_json}")
        if res.per_core_scope_times:
            for kname, v in sorted(res.per_core_scope_times.items()):
                print(f"  scope {kname}: {v}")
    losses = np.concatenate([r["out"].reshape(-1) for r in res.results])
    loss = losses.mean() + (H * H) * (EPS * EPS)
    return np.float32(WA * loss)
